# revision 2
# baseline (speedup 1.0000x reference)
"""GATConv (multi-head graph attention) on 8 Trainium2 NeuronCores — v2.

Same contract as the baseline kernel: kernel(**inputs) takes FULL numpy
inputs, returns FULL [50000, 256] f32 output.  Distribution: nodes
block-partitioned across 8 cores.

v2 changes vs baseline (which measured 2.85ms):
 - window gathers (g2/gs: 200k SWDGE descriptors/core) replaced by eqT
   selector matmuls on the Tensor engine (eqT tables shipped like eq).
 - 2 dma_gather calls per block (lo/hi) on rotating SWDGE queues instead
   of 8 small calls.
 - phase-0 projection in bf16 instead of fp32r (4x PE throughput).
 - per-block batched DVE/ACT ops; per-(tile,head) tensor_scalar message
   multiply (per-partition scalar = alpha column).
"""

import sys

sys.path.insert(0, "/opt/trn_rl_repo")

import numpy as np

N_NODES = 50000
N_EDGES = 800000
IN_DIM = 512
HEADS = 4
OUT_DIM = 64
F = HEADS * OUT_DIM  # 256
N_CORES = 8
HALF = 32768  # int16 gather index range split
GROW = 384  # G table bf16 elems/row: 256 h | 8 (s_src f32 bits) | 8 (recip f32 bits) | pad
SROW = 64  # S table f32 elems/row: 4 s_src | 4 s_dst | pad -> 256B
H2 = 2 * HEADS


def _ceil_div(a, b):
    return (a + b - 1) // b


def _wrap16(arr_i16):
    """dma_gather idx layout: position i -> [i % 16, i // 16], x8 core groups."""
    n = arr_i16.shape[0]
    assert n % 16 == 0
    w = arr_i16.reshape(n // 16, 16).T
    return np.ascontiguousarray(np.tile(w, (8, 1)))


def _build_phase_meta(key, other, rel, n_cores, nblk, half):
    """Per-core gather indices + bf16 eq/eqT selector matrices for one pass.

    Edges grouped by key-node block (128 nodes); within a block a low run
    (other < half) and a high run, each padded to a multiple of 128.
    Tile counts (T_lo, T_hi) uniform across blocks/cores (same program).

    Returns (T_lo, T_hi, gidx[c], eq[c], eqT[c]):
      gidx: [128, nblk*T*8] int16  gather idx into half table (pad -> 0)
      eq:   [nblk*T*128, 128] bf16 selector: eq[lane, j] = (key_rel == j)
      eqT:  [nblk*T*128, 128] bf16 transposed: eqT[j, lane] = (key_rel == j)
    """
    import ml_dtypes

    gblk = key >> 7
    hi = (other >= half).astype(np.int64)
    run = gblk * 2 + hi
    order = np.lexsort((other, run))
    run_s = run[order]
    other_s = other[order]
    rel_s = rel[order]

    nruns = n_cores * nblk * 2
    counts = np.bincount(run_s, minlength=nruns)
    T_lo = max(1, int(np.max(_ceil_div(counts[0::2], 128))))
    T_hi = int(np.max(_ceil_div(counts[1::2], 128)))
    T = T_lo + T_hi

    starts = np.zeros(nruns, np.int64)
    np.cumsum(counts[:-1], out=starts[1:])
    rank = np.arange(len(key), dtype=np.int64) - starts[run_s]
    pos = (run_s >> 1) * (T * 128) + hi[order] * (T_lo * 128) + rank

    total = n_cores * nblk * T * 128
    gidx = np.zeros(total, np.int16)
    gidx[pos] = (other_s - (other_s >= half) * half).astype(np.int16)
    eq = np.zeros((total, 128), ml_dtypes.bfloat16)
    eq[pos, rel_s] = 1.0
    eqT = np.zeros((total, 128), ml_dtypes.bfloat16)
    eqT[(pos >> 7 << 7) + rel_s, pos & 127] = 1.0

    per_core = nblk * T * 128
    gidx_c, eq_c, eqT_c = [], [], []
    for c in range(n_cores):
        sl = slice(c * per_core, (c + 1) * per_core)
        gidx_c.append(_wrap16(gidx[sl]))
        # partition-major [128, nblk*T*128]: row l holds all tiles' lane-l
        # selector rows contiguously -> per-block loads are 1 big descriptor
        # per partition instead of nblk*T strided 256B ones.
        eq_c.append(np.ascontiguousarray(
            eq[sl].reshape(nblk * T, 128, 128).transpose(1, 0, 2)
            .reshape(128, nblk * T * 128)))
        eqT_c.append(np.ascontiguousarray(
            eqT[sl].reshape(nblk * T, 128, 128).transpose(1, 0, 2)
            .reshape(128, nblk * T * 128)))
    return T_lo, T_hi, gidx_c, eq_c, eqT_c


def _build_bass_program(npad, rpc, nblk, t1_lo, t1_hi, t2_lo, t2_hi, n_cores,
                        half, enable_asserts=False):
    import concourse.bacc as bacc
    import concourse.mybir as mybir
    import concourse.tile as tile

    dt = mybir.dt
    Alu = mybir.AluOpType
    Act = mybir.ActivationFunctionType
    T1 = t1_lo + t1_hi
    T2 = t2_lo + t2_hi
    KC = IN_DIM // 128
    WCOL = F + H2  # 264
    f32r = dt.float32r
    bf16 = dt.bfloat16

    nc = bacc.Bacc(
        "TRN2",
        target_bir_lowering=False,
        debug=False,
        enable_asserts=enable_asserts,
        num_devices=n_cores,
        num_swdge_queues=4,
    )

    xT = nc.dram_tensor("xT", [IN_DIM, rpc], dt.float32, kind="ExternalInput")
    W_in = nc.dram_tensor("W", [IN_DIM, F], dt.float32, kind="ExternalInput")
    A0_in = nc.dram_tensor("A0", [128, H2], dt.float32, kind="ExternalInput")
    A1_in = nc.dram_tensor("A1", [128, H2], dt.float32, kind="ExternalInput")
    bias_in = nc.dram_tensor("bias", [1, F], dt.float32, kind="ExternalInput")
    p1_gidx = nc.dram_tensor("p1_gidx", [128, nblk * T1 * 8], dt.int16, kind="ExternalInput")
    p1_eq = nc.dram_tensor("p1_eq", [128, nblk * T1 * 128], bf16, kind="ExternalInput")
    p1_eqT = nc.dram_tensor("p1_eqT", [128, nblk * T1 * 128], bf16, kind="ExternalInput")
    p2_gidx = nc.dram_tensor("p2_gidx", [128, nblk * T2 * 8], dt.int16, kind="ExternalInput")
    p2_eq = nc.dram_tensor("p2_eq", [128, nblk * T2 * 128], bf16, kind="ExternalInput")
    p2_eqT = nc.dram_tensor("p2_eqT", [128, nblk * T2 * 128], bf16, kind="ExternalInput")
    out = nc.dram_tensor("out", [rpc, F], dt.float32, kind="ExternalOutput")

    with tile.TileContext(nc) as tc:
        with (
            tc.tile_pool(name="const", bufs=1) as cpool,
            tc.tile_pool(name="dram", bufs=1, space="DRAM") as dram,
        ):
            G_own = dram.tile([rpc, GROW], bf16)
            S_own = dram.tile([rpc, HEADS], dt.float32)
            G_full = dram.tile([npad, GROW], bf16, addr_space="Shared")
            S_cfull = dram.tile([npad, HEADS], dt.float32, addr_space="Shared")
            S_full = dram.tile([npad, SROW], dt.float32, addr_space="Shared")

            # ---------------- constants ----------------
            iota_i = cpool.tile([128, 128], dt.int32)
            nc.gpsimd.iota(iota_i[:], pattern=[[1, 128]], channel_multiplier=0)
            iota_f = cpool.tile([128, 128], dt.float32)
            nc.vector.tensor_copy(iota_f[:], iota_i[:])
            pidx_i = cpool.tile([128, 1], dt.int32)
            nc.gpsimd.iota(pidx_i[:], pattern=[[0, 1]], channel_multiplier=1)
            pidx_f = cpool.tile([128, 1], dt.float32)
            nc.vector.tensor_copy(pidx_f[:], pidx_i[:])
            ident = cpool.tile([128, 128], dt.float32)
            nc.vector.tensor_scalar(ident[:], iota_f[:], pidx_f[:], None, op0=Alu.is_equal)

            bias_bc = cpool.tile([128, F], dt.float32)
            nc.sync.dma_start(bias_bc[:1, :], bias_in[:, :])
            nc.gpsimd.partition_broadcast(bias_bc[:], bias_bc[:1, :])

            A0 = cpool.tile([128, H2], dt.float32)
            A1 = cpool.tile([128, H2], dt.float32)
            nc.sync.dma_start(A0[:], A0_in[:])
            nc.sync.dma_start(A1[:], A1_in[:])

            W_sb = cpool.tile([128, KC * WCOL], dt.float32)
            for kc in range(KC):
                nc.sync.dma_start(
                    W_sb[:, kc * WCOL:kc * WCOL + F], W_in[kc * 128:(kc + 1) * 128, :]
                )
            WT0 = cpool.tile([128, IN_DIM], dt.float32)
            WT1 = cpool.tile([128, IN_DIM], dt.float32)
            with tc.tile_pool(name="psum_pre", bufs=2, space="PSUM") as pp:
                for kc in range(KC):
                    for fc in range(2):
                        pt = pp.tile([128, 128], dt.float32, tag="tr")
                        nc.tensor.transpose(
                            pt[:],
                            W_sb[:, kc * WCOL + fc * 128:kc * WCOL + (fc + 1) * 128],
                            ident[:],
                        )
                        wt = WT0 if fc == 0 else WT1
                        nc.vector.tensor_copy(wt[:, kc * 128:(kc + 1) * 128], pt[:])
                for kc in range(KC):
                    pwa = pp.tile([128, H2], dt.float32, tag="wa")
                    for fc in range(2):
                        wt = WT0 if fc == 0 else WT1
                        A = A0 if fc == 0 else A1
                        nc.tensor.matmul(
                            pwa[:], wt[:, kc * 128:(kc + 1) * 128], A[:],
                            start=(fc == 0), stop=(fc == 1),
                        )
                    nc.vector.tensor_copy(W_sb[:, kc * WCOL + F:(kc + 1) * WCOL], pwa[:])
            W_sbr = cpool.tile([128, KC * WCOL], bf16)
            nc.vector.tensor_copy(W_sbr[:], W_sb[:])

            # s values of the core's own nodes, resident (bf16 for eqT matmuls)
            s_res = cpool.tile([128, nblk * H2], bf16)
            rec_all = cpool.tile([128, nblk * HEADS], dt.float32)

            # ---------------- phase 0: projection (bf16) ----------------
            with (
                tc.tile_pool(name="p0", bufs=3) as p0pool,
                tc.tile_pool(name="p0ps", bufs=2, space="PSUM") as p0ps,
            ):
                BB = 2  # node blocks per iteration
                for r0 in range(0, nblk, BB):
                    nb = min(BB, nblk - r0)
                    xt = p0pool.tile([128, KC, BB * 128], dt.float32, tag="xt")
                    nc.sync.dma_start(
                        xt[:, :, 0:nb * 128],
                        xT[:].rearrange("(k p) n -> p k n", p=128)[
                            :, :, r0 * 128:(r0 + nb) * 128],
                    )
                    xtr = p0pool.tile([128, KC, BB * 128], bf16, tag="xtr")
                    nc.vector.tensor_copy(xtr[:, :, 0:nb * 128], xt[:, :, 0:nb * 128])
                    gsb = p0pool.tile([128, BB, GROW], bf16, tag="gsb")
                    ssb = p0pool.tile([128, BB, HEADS], dt.float32, tag="ssb")
                    nc.vector.memset(gsb[:, :, F + H2:GROW], 0.0)
                    for c in range(nb):
                        r = r0 + c
                        ps = p0ps.tile([128, WCOL], dt.float32, tag="hps")
                        for kc in range(KC):
                            nc.tensor.matmul(
                                ps[:], xtr[:, kc, c * 128:(c + 1) * 128],
                                W_sbr[:, kc * WCOL:(kc + 1) * WCOL],
                                start=(kc == 0), stop=(kc == KC - 1),
                            )
                        nc.vector.tensor_copy(gsb[:, c, 0:F], ps[:, 0:F])
                        nc.vector.tensor_copy(  # s_src raw f32 bits
                            gsb[:, c, F:F + H2].bitcast(dt.float32),
                            ps[:, F:F + HEADS]
                        )
                        nc.vector.tensor_copy(ssb[:, c, :], ps[:, F + HEADS:WCOL])
                        nc.vector.tensor_copy(
                            s_res[:, r * H2:(r + 1) * H2], ps[:, F:WCOL]
                        )
                    nc.sync.dma_start(
                        G_own[r0 * 128:(r0 + nb) * 128, :].rearrange(
                            "(c p) g -> p c g", p=128),
                        gsb[:, 0:nb, :])
                    nc.sync.dma_start(
                        S_own[r0 * 128:(r0 + nb) * 128, :].rearrange(
                            "(c p) g -> p c g", p=128),
                        ssb[:, 0:nb, :])

            nc.gpsimd.collective_compute(
                "AllGather", Alu.bypass,
                ins=[S_own[:].opt()], outs=[S_cfull[:].opt()],
                replica_groups=[list(range(n_cores))],
            )
            # expand compact s_dst into 256B-stride gatherable rows
            nc.sync.dma_start(S_full[:, 0:HEADS], S_cfull[:])

            # ---------------- phase 1: softmax denominators ----------------
            with (
                tc.tile_pool(name="p1idx", bufs=1) as p1i,
                tc.tile_pool(name="p1", bufs=6) as p1pool,
                tc.tile_pool(name="p1ps", bufs=2, space="PSUM") as p1ps,
            ):
                gidx_sb = p1i.tile([128, nblk * T1 * 8], dt.int16)
                nc.sync.dma_start(gidx_sb[:], p1_gidx[:])

                for b in range(nblk):
                    co = b * T1 * 8
                    # 4 gather calls per block across all queues, each into
                    # its own tile so the writes are independent (lo split
                    # in 2, hi split in 2; queue map flips per block parity)
                    la = (t1_lo + 1) // 2
                    ha = (t1_hi + 1) // 2
                    S_lo = S_full[0:half, :] if npad > half else S_full[:, :]
                    calls = [(0, la, S_lo), (la, t1_lo - la, S_lo)]
                    if t1_hi:
                        calls += [(t1_lo, ha, S_full[half:npad, :]),
                                  (t1_lo + ha, t1_hi - ha, S_full[half:npad, :])]
                    g1p = []
                    for i, (t0, ntl, src_ap) in enumerate(calls):
                        if ntl <= 0:
                            continue
                        gt = p1pool.tile([128, ntl, SROW], dt.float32,
                                         tag=f"g1_{i}")
                        g1p.append((t0, ntl, gt))
                        nc.gpsimd.dma_gather(
                            gt[:], src_ap,
                            gidx_sb[:, co + t0 * 8:co + (t0 + ntl) * 8],
                            ntl * 128, ntl * 128, SROW,
                            single_packet=False,
                            queue_num=(i + 2 * (b % 2)) % 4,
                        )
                    eq_sb = p1pool.tile([128, T1 * 128], bf16, tag="eqs1")
                    nc.sync.dma_start(
                        eq_sb[:], p1_eq[:, b * T1 * 128:(b + 1) * T1 * 128])
                    eqT_sb = p1pool.tile([128, T1 * 128], bf16, tag="eqTs1")
                    nc.sync.dma_start(
                        eqT_sb[:], p1_eqT[:, b * T1 * 128:(b + 1) * T1 * 128])

                    # s_src per edge via eqT matmul: slab[:, t*4:(t+1)*4] =
                    # eqT_t^T @ s_src_blk   (lane layout)
                    slab = p1ps.tile([128, T1 * HEADS], dt.float32, tag="slab1")
                    for t in range(T1):
                        nc.tensor.matmul(
                            slab[:, t * HEADS:(t + 1) * HEADS],
                            eqT_sb[:, t * 128:(t + 1) * 128],
                            s_res[:, b * H2:b * H2 + HEADS],
                            start=True, stop=True,
                        )
                    # z = s_src_edge + s_dst_gathered ; lrelu ; exp (bf16)
                    z = p1pool.tile([128, T1 * HEADS], dt.float32, tag="z1")
                    for t0, ntl, gt in g1p:
                        nc.vector.tensor_tensor(
                            z[:, t0 * HEADS:(t0 + ntl) * HEADS].rearrange(
                                "p (t h) -> p t h", h=HEADS),
                            slab[:, t0 * HEADS:(t0 + ntl) * HEADS].rearrange(
                                "p (t h) -> p t h", h=HEADS),
                            gt[:, :, 0:HEADS], op=Alu.add
                        )
                    nc.vector.scalar_tensor_tensor(
                        z[:], z[:], 0.2, z[:], op0=Alu.mult, op1=Alu.max
                    )
                    v = p1pool.tile([128, T1 * HEADS], bf16, tag="v1")
                    nc.scalar.activation(v[:], z[:], Act.Exp)

                    # sumexp^T [4, 128] += v_t^T @ EQ_t  (v stationary: 4 cols)
                    ps1 = p1ps.tile([4, 128], dt.float32, tag="ps1")
                    for t in range(T1):
                        nc.tensor.matmul(
                            ps1[:], v[:, t * HEADS:(t + 1) * HEADS],
                            eq_sb[:, t * 128:(t + 1) * 128],
                            start=(t == 0), stop=(t == T1 - 1),
                        )
                    se_sb = p1pool.tile([4, 128], dt.float32, tag="se")
                    nc.vector.tensor_copy(se_sb[:], ps1[:])
                    ps_tr = p1ps.tile([128, 4], dt.float32, tag="ps1t")
                    nc.tensor.transpose(ps_tr[:], se_sb[:], ident[0:4, 0:4])
                    nc.vector.tensor_scalar_add(
                        rec_all[:, b * HEADS:(b + 1) * HEADS], ps_tr[:], 1e-10
                    )
                recd = cpool.tile([128, nblk * HEADS], dt.float32)
                nc.vector.reciprocal(recd[:], rec_all[:])
                for b in range(nblk):
                    nc.sync.dma_start(
                        G_own[b * 128:(b + 1) * 128,
                              F + H2:F + 2 * H2].bitcast(dt.float32),
                        recd[:, b * HEADS:(b + 1) * HEADS],
                    )

            nc.gpsimd.collective_compute(
                "AllGather", Alu.bypass,
                ins=[G_own[:].opt()], outs=[G_full[:].opt()],
                replica_groups=[list(range(n_cores))],
            )

            # ---------------- phase 2: aggregate messages ----------------
            with (
                tc.tile_pool(name="p2idx", bufs=1) as p2i,
                tc.tile_pool(name="p2", bufs=6) as p2pool,
                tc.tile_pool(name="p2ps", bufs=3, space="PSUM") as p2ps,
            ):
                gidx2_sb = p2i.tile([128, nblk * T2 * 8], dt.int16)
                nc.sync.dma_start(gidx2_sb[:], p2_gidx[:])

                for b in range(nblk):
                    co = b * T2 * 8
                    la = (t2_lo + 1) // 2
                    ha = (t2_hi + 1) // 2
                    G_lo = G_full[0:half, :] if npad > half else G_full[:, :]
                    calls = [(0, la, G_lo), (la, t2_lo - la, G_lo)]
                    if t2_hi:
                        calls += [(t2_lo, ha, G_full[half:npad, :]),
                                  (t2_lo + ha, t2_hi - ha, G_full[half:npad, :])]
                    gp = []
                    for i, (t0, ntl, src_ap) in enumerate(calls):
                        if ntl <= 0:
                            continue
                        gt = p2pool.tile([128, ntl, GROW], bf16, tag=f"g_{i}")
                        gp.append((t0, ntl, gt))
                        nc.gpsimd.dma_gather(
                            gt[:], src_ap,
                            gidx2_sb[:, co + t0 * 8:co + (t0 + ntl) * 8],
                            ntl * 128, ntl * 128, GROW,
                            single_packet=False,
                            queue_num=(i + 2 * (b % 2)) % 4,
                        )
                    eq_sb = p2pool.tile([128, T2 * 128], bf16, tag="eqs2")
                    nc.sync.dma_start(
                        eq_sb[:], p2_eq[:, b * T2 * 128:(b + 1) * T2 * 128])
                    eqT_sb = p2pool.tile([128, T2 * 128], bf16, tag="eqTs2")
                    nc.sync.dma_start(
                        eqT_sb[:], p2_eqT[:, b * T2 * 128:(b + 1) * T2 * 128])

                    # s_dst per edge via eqT matmuls into psum slab
                    slab = p2ps.tile([128, T2 * HEADS], dt.float32, tag="slab2")
                    for t in range(T2):
                        nc.tensor.matmul(
                            slab[:, t * HEADS:(t + 1) * HEADS],
                            eqT_sb[:, t * 128:(t + 1) * 128],
                            s_res[:, b * H2 + HEADS:(b + 1) * H2],
                            start=True, stop=True,
                        )
                    # alpha = exp(leaky_relu(s_src + s_dst)) * recip
                    al = p2pool.tile([128, T2 * HEADS], dt.float32, tag="al")
                    for t0, ntl, gt in gp:
                        nc.vector.tensor_tensor(
                            al[:, t0 * HEADS:(t0 + ntl) * HEADS].rearrange(
                                "p (t h) -> p t h", h=HEADS),
                            slab[:, t0 * HEADS:(t0 + ntl) * HEADS].rearrange(
                                "p (t h) -> p t h", h=HEADS),
                            gt[:, :, F:F + H2].bitcast(dt.float32),
                            op=Alu.add,
                        )
                    nc.vector.scalar_tensor_tensor(
                        al[:], al[:], 0.2, al[:], op0=Alu.mult, op1=Alu.max
                    )
                    nc.scalar.activation(al[:], al[:], Act.Exp)
                    for t0, ntl, gt in gp:
                        nc.vector.tensor_tensor(
                            al[:, t0 * HEADS:(t0 + ntl) * HEADS].rearrange(
                                "p (t h) -> p t h", h=HEADS),
                            al[:, t0 * HEADS:(t0 + ntl) * HEADS].rearrange(
                                "p (t h) -> p t h", h=HEADS),
                            gt[:, :, F + H2:F + 2 * H2].bitcast(dt.float32),
                            op=Alu.mult,
                        )
                    ps2 = p2ps.tile([128, F], dt.float32, tag="ps2")
                    for t0, ntl, gt in gp:
                        for tt in range(ntl):
                            t = t0 + tt
                            alpha_b = al[:, t * HEADS:(t + 1) * HEADS].unsqueeze(
                                1
                            ).broadcast_to([128, OUT_DIM, HEADS])
                            msg = p2pool.tile([128, F], bf16, tag="msg")
                            nc.vector.tensor_tensor(
                                msg[:].rearrange("p (d h) -> p d h", h=HEADS),
                                gt[:, tt, 0:F].rearrange("p (d h) -> p d h",
                                                         h=HEADS),
                                alpha_b,
                                op=Alu.mult,
                            )
                            nc.tensor.matmul(
                                ps2[:], eq_sb[:, t * 128:(t + 1) * 128], msg[:],
                                start=(t == 0), stop=(t == T2 - 1),
                            )
                    osb = p2pool.tile([128, F], dt.float32, tag="osb")
                    nc.vector.tensor_tensor(osb[:], ps2[:], bias_bc[:], op=Alu.add)
                    nc.sync.dma_start(out[b * 128:(b + 1) * 128, :], osb[:])

    nc.compile()
    return nc


def _gat_forward(x, edges, W, a, bias, n_nodes, n_cores, half=HALF,
                 run_opts=None):
    npad = _ceil_div(n_nodes, n_cores * 128) * n_cores * 128
    rpc = npad // n_cores
    nblk = rpc // 128

    src = edges[:, 0].astype(np.int64)
    dst = edges[:, 1].astype(np.int64)
    t1_lo, t1_hi, p1_gidx, p1_eq, p1_eqT = _build_phase_meta(
        src, dst, src % 128, n_cores, nblk, half
    )
    t2_lo, t2_hi, p2_gidx, p2_eq, p2_eqT = _build_phase_meta(
        dst, src, dst % 128, n_cores, nblk, half
    )

    nc = _build_bass_program(npad, rpc, nblk, t1_lo, t1_hi, t2_lo, t2_hi,
                             n_cores, half)

    x_pad = np.zeros((npad, IN_DIM), np.float32)
    x_pad[:n_nodes] = x
    xT = np.ascontiguousarray(x_pad.T)

    # d-major head layout: device feature column f = d*HEADS+h holds
    # logical column h*OUT_DIM+d
    dev_from_log = np.arange(F).reshape(HEADS, OUT_DIM).T.reshape(-1)
    W_p = np.ascontiguousarray(W.astype(np.float32)[:, dev_from_log])
    bias_p = np.ascontiguousarray(
        bias.astype(np.float32)[dev_from_log].reshape(1, F))
    # A selector matrices: partition = device feature col within its
    # 128-chunk (A0: cols 0:128, A1: cols 128:256)
    A0 = np.zeros((128, H2), np.float32)
    A1 = np.zeros((128, H2), np.float32)
    af = a.astype(np.float32)
    for h in range(HEADS):
        for d in range(OUT_DIM):
            f = d * HEADS + h
            dstA, row = (A0, f) if f < 128 else (A1, f - 128)
            dstA[row, h] = af[h, d]
            dstA[row, HEADS + h] = af[h, OUT_DIM + d]

    in_maps = []
    for c in range(n_cores):
        in_maps.append({
            "xT": np.ascontiguousarray(xT[:, c * rpc:(c + 1) * rpc]),
            "W": W_p, "A0": A0, "A1": A1, "bias": bias_p,
            "p1_gidx": p1_gidx[c], "p1_eq": p1_eq[c], "p1_eqT": p1_eqT[c],
            "p2_gidx": p2_gidx[c], "p2_eq": p2_eq[c], "p2_eqT": p2_eqT[c],
        })

    from concourse.bass_utils import run_bass_kernel_spmd

    res = run_bass_kernel_spmd(
        nc, in_maps, core_ids=list(range(n_cores)), **(run_opts or {})
    )
    out = np.concatenate([r["out"] for r in res.results], axis=0)
    out = out[:, dev_from_log.argsort()]  # back to h-major logical layout
    return np.ascontiguousarray(out[:n_nodes]), res


def kernel(x, edges, W, a, bias):
    x = np.asarray(x, np.float32)
    edges = np.asarray(edges)
    W = np.asarray(W, np.float32)
    a = np.asarray(a, np.float32)
    bias = np.asarray(bias, np.float32)
    out, _ = _gat_forward(x, edges, W, a, bias, N_NODES, N_CORES)
    return out
